# revision 17
# baseline (speedup 1.0000x reference)
"""MoE expert-combine kernel for Trainium2 (raw Bass, hand-scheduled), 8-core SPMD.

Problem: out[b,s,:] = sum_k expert_weights[b,s,k] * expert_outputs[expert_indices[b,s,k], b, s, :]
  B,S,H = 4,2048,1024 ; E=8 ; K=2  (hidden_states is unused by the reference)

Sharding: flatten tokens t = b*S+s (8192 total); each of the 8 cores owns a
contiguous block of 1024 tokens. Each core receives the expert-output stack
sliced to its tokens ([E, 1024, H] viewed as a row table [E*1024, H]) plus
host-precomputed gather row indices (idx[t,k]*1024 + t_local, int32
[128, 16]) and weights (f32 [128, 16]).

On-device, per 128-token chunk c (token = c*128 + p): two indirect-DMA
gathers (Pool/SWDGE) fetch the selected 4KB table rows, DVE combines
(w0*g0 via tensor_scalar, then (w1*g1)+acc via scalar_tensor_tensor), and an
HWDGE store writes [128, 1024] back. The row-index tensor is loaded in its
own small DMA ahead of the weights so the first gather can start as early as
possible. Hand-placed semaphores, at most one sync-wait per compute
instruction (walrus codegen limit), and no end-of-block drain/barrier (the
sync engine's final sem_st wait covers every data dependency; the NEFF's own
per-engine completion chain runs regardless).
"""

import sys
import numpy as np

for _p in ("/opt/trn_rl_repo", "/opt/pypackages"):
    if _p not in sys.path:
        sys.path.append(_p)

from concourse import bass, mybir
from concourse.bass_utils import run_bass_kernel_spmd

B, S, H = 4, 2048, 1024
E, K = 8, 2
N_CORES = 8
T = B * S              # 8192 tokens total
TC = T // N_CORES      # 1024 tokens per core
P = 128                # SBUF partitions
NCHUNK = TC // P       # 8 chunks of 128 tokens per core

_f32 = mybir.dt.float32
_i32 = mybir.dt.int32


def _build():
    nc = bass.Bass(target_bir_lowering=False)

    table = nc.declare_dram_parameter("table", [E * TC, H], _f32, isOutput=False)
    idx = nc.declare_dram_parameter("idx", [P, NCHUNK * K], _i32, isOutput=False)
    wgt = nc.declare_dram_parameter("wgt", [P, NCHUNK * K], _f32, isOutput=False)
    out = nc.declare_dram_parameter("out", [TC, H], _f32, isOutput=True)

    with (
        nc.semaphore("sem_idx") as sem_idx,
        nc.semaphore("sem_w") as sem_w,
        nc.semaphore("sem_v") as sem_v,
        nc.semaphore("sem_st") as sem_st,
        nc.sbuf_tensor("idx_t", [P, NCHUNK * K], _i32) as idx_t,
        nc.sbuf_tensor("w_t", [P, NCHUNK * K], _f32) as w_t,
        nc.sbuf_tensor("g_t", [P, NCHUNK * K * H], _f32) as g_t,
        nc.sbuf_tensor("ot_t", [P, NCHUNK * H], _f32) as ot_t,
        nc.sbuf_tensor("acc_t", [P, H], _f32) as acc_t,
    ):
        gather_sems = [nc.alloc_semaphore(f"sem_g{i}") for i in range(NCHUNK * K)]
        blk = bass.BassBlock(nc, "main")
        nc.cur_block = blk

        def sync_body(sync: bass.BassEngine):
            sync.dma_start(out=idx_t[:], in_=idx[:]).then_inc(sem_idx, 16)
            sync.dma_start(out=w_t[:], in_=wgt[:]).then_inc(sem_w, 16)
            for c in range(NCHUNK):
                # ot chunk c is ready after DVE op 2c+2 (1 sem inc per op)
                sync.wait_ge(sem_v, 2 * c + 2)
                sync.dma_start(
                    out=out[c * P : (c + 1) * P, :],
                    in_=ot_t[:, c * H : (c + 1) * H],
                ).then_inc(sem_st, 16)
            sync.wait_ge(sem_st, 16 * NCHUNK)

        def gpsimd_body(gpsimd: bass.BassEngine):
            gpsimd.wait_ge(sem_idx, 16)
            for c in range(NCHUNK):
                for k in range(K):
                    m = c * K + k
                    gpsimd.indirect_dma_start(
                        out=g_t[:, m * H : (m + 1) * H],
                        out_offset=None,
                        in_=table[:],
                        in_offset=bass.IndirectOffsetOnAxis(
                            ap=idx_t[:, m : m + 1], axis=0
                        ),
                    ).then_inc(gather_sems[m], 16)

        def vector_body(vector: bass.BassEngine):
            # one-time gate on the weight load; afterwards each op's single
            # wait slot is spent on its gather sem
            vector.wait_ge(sem_w, 16)
            for c in range(NCHUNK):
                m0, m1 = c * K, c * K + 1
                w0 = w_t[:, m0 : m0 + 1]
                w1 = w_t[:, m1 : m1 + 1]
                vector.tensor_scalar(
                    out=acc_t[:],
                    in0=g_t[:, m0 * H : (m0 + 1) * H],
                    scalar1=w0,
                    scalar2=None,
                    op0=mybir.AluOpType.mult,
                )._wait_ge(gather_sems[m0], 16).then_inc(sem_v, 1)
                vector.scalar_tensor_tensor(
                    out=ot_t[:, c * H : (c + 1) * H],
                    in0=g_t[:, m1 * H : (m1 + 1) * H],
                    scalar=w1,
                    in1=acc_t[:],
                    op0=mybir.AluOpType.mult,
                    op1=mybir.AluOpType.add,
                )._wait_ge(gather_sems[m1], 16).then_inc(sem_v, 1)

        blk.sync(sync_body)
        blk.gpsimd(gpsimd_body)
        blk.vector(vector_body)

        # Manual block exit WITHOUT the end-of-block drains + all-engine
        # barrier: branch every engine straight to the end block.
        for engine, last_body in blk.last_body.items():
            with nc.body(last_body, parent=nc.cur_bb, allow_existing_parent=True):
                engine.br(blk.end_bb)
        nc.switch_bb(blk.end_bb)
        nc.cur_block = None

    # Strip the preamble's const-tile memsets and the post-init all-engine
    # barrier (~2.5us): this kernel never reads the const APs, and each
    # engine's register init precedes its user code in program order anyway.
    entry = nc.m.functions[0].blocks[0]
    drop = {
        ins.name
        for ins in entry.instructions
        if type(ins).__name__
        in ("InstMemset", "InstDrain", "InstEventSemaphore", "InstRegisterMove")
    }
    kept = [ins for ins in entry.instructions if ins.name not in drop]
    del entry.instructions[:]
    for ins in kept:
        entry.instructions.append(ins)

    nc.finalize()
    return nc


def _prepare_in_maps(expert_indices, expert_weights, expert_outputs):
    eo = np.ascontiguousarray(np.asarray(expert_outputs, dtype=np.float32)).reshape(
        E, T, H
    )
    flat_idx = np.asarray(expert_indices).reshape(T, K).astype(np.int32)
    flat_w = np.asarray(expert_weights, dtype=np.float32).reshape(T, K)
    t_local = np.arange(TC, dtype=np.int32)[:, None]
    in_maps = []
    for i in range(N_CORES):
        t0 = i * TC
        slab = np.ascontiguousarray(eo[:, t0 : t0 + TC, :]).reshape(E * TC, H)
        li = flat_idx[t0 : t0 + TC] * TC + t_local  # [TC, K] row idx into slab
        # chunk-major: partition p of chunk c holds token c*128+p
        li = np.ascontiguousarray(
            li.reshape(NCHUNK, P, K).transpose(1, 0, 2).reshape(P, NCHUNK * K)
        )
        w = np.ascontiguousarray(
            flat_w[t0 : t0 + TC]
            .reshape(NCHUNK, P, K)
            .transpose(1, 0, 2)
            .reshape(P, NCHUNK * K)
            .astype(np.float32)
        )
        in_maps.append({"table": slab, "idx": li, "wgt": w})
    return in_maps


def run(
    hidden_states,
    expert_indices,
    expert_weights,
    expert_outputs,
    trace=False,
):
    in_maps = _prepare_in_maps(expert_indices, expert_weights, expert_outputs)
    nc = _build()
    res = run_bass_kernel_spmd(nc, in_maps, list(range(N_CORES)), trace=trace)
    outs = [np.asarray(res.results[i]["out"]) for i in range(N_CORES)]
    full = np.concatenate(outs, axis=0).reshape(B, S, H).astype(np.float32)
    return full, res


def kernel(hidden_states, expert_indices, expert_weights, expert_outputs):
    full, _ = run(hidden_states, expert_indices, expert_weights, expert_outputs)
    return full


# revision 19
# speedup vs baseline: 1.1805x; 1.1805x over previous
"""MoE expert-combine kernel for Trainium2 (raw Bass, hand-scheduled), 8-core SPMD.

Problem: out[b,s,:] = sum_k expert_weights[b,s,k] * expert_outputs[expert_indices[b,s,k], b, s, :]
  B,S,H = 4,2048,1024 ; E=8 ; K=2  (hidden_states is unused by the reference)

Sharding: flatten tokens t = b*S+s (8192 total); each of the 8 cores owns a
contiguous block of 1024 tokens. Each core receives the expert-output stack
sliced to its tokens ([E, 1024, H] viewed as a row table [E*1024, H]) plus
host-precomputed gather row indices (idx[t,k]*1024 + t_local, int32
[128, 16]) and weights (f32 [128, 16]).

On-device, per 128-token chunk c (token = c*128 + p): two indirect-DMA
gathers (Pool/SWDGE) fetch the selected 4KB table rows, DVE combines
(w0*g0 via tensor_scalar, then (w1*g1)+acc via scalar_tensor_tensor), and an
HWDGE store writes [128, 1024] back. The row-index tensor is loaded in its
own small DMA ahead of the weights so the first gather can start as early as
possible. Hand-placed semaphores, at most one sync-wait per compute
instruction (walrus codegen limit), and no end-of-block drain/barrier (the
sync engine's final sem_st wait covers every data dependency; the NEFF's own
per-engine completion chain runs regardless).
"""

import sys
import numpy as np

for _p in ("/opt/trn_rl_repo", "/opt/pypackages"):
    if _p not in sys.path:
        sys.path.append(_p)

from concourse import bass, mybir
from concourse.bass_utils import run_bass_kernel_spmd

B, S, H = 4, 2048, 1024
E, K = 8, 2
N_CORES = 8
T = B * S              # 8192 tokens total
TC = T // N_CORES      # 1024 tokens per core
P = 128                # SBUF partitions
NCHUNK = TC // P       # 8 chunks of 128 tokens per core

_f32 = mybir.dt.float32
_i32 = mybir.dt.int32


def _build():
    nc = bass.Bass(target_bir_lowering=False)

    table = nc.declare_dram_parameter("table", [E * TC, H], _f32, isOutput=False)
    idx = nc.declare_dram_parameter("idx", [P, NCHUNK * K], _i32, isOutput=False)
    wgt = nc.declare_dram_parameter("wgt", [P, NCHUNK * K], _f32, isOutput=False)
    out = nc.declare_dram_parameter("out", [TC, H], _f32, isOutput=True)

    with (
        nc.semaphore("sem_idx") as sem_idx,
        nc.semaphore("sem_w") as sem_w,
        nc.semaphore("sem_v") as sem_v,
        nc.semaphore("sem_st") as sem_st,
        nc.sbuf_tensor("idx_t", [P, NCHUNK * K], _i32) as idx_t,
        nc.sbuf_tensor("w_t", [P, NCHUNK * K], _f32) as w_t,
        nc.sbuf_tensor("g_t", [P, NCHUNK * K * H], _f32) as g_t,
        nc.sbuf_tensor("ot_t", [P, NCHUNK * H], _f32) as ot_t,
        nc.sbuf_tensor("acc_t", [P, H], _f32) as acc_t,
    ):
        gather_sems = [nc.alloc_semaphore(f"sem_g{i}") for i in range(NCHUNK * K)]

        def sync_body(sync: bass.BassEngine):
            sync.dma_start(out=idx_t[:], in_=idx[:]).then_inc(sem_idx, 16)
            sync.dma_start(out=w_t[:], in_=wgt[:]).then_inc(sem_w, 16)
            for c in range(NCHUNK):
                # ot chunk c is ready after DVE op 2c+2 (1 sem inc per op)
                sync.wait_ge(sem_v, 2 * c + 2)
                sync.dma_start(
                    out=out[c * P : (c + 1) * P, :],
                    in_=ot_t[:, c * H : (c + 1) * H],
                ).then_inc(sem_st, 16)
            sync.wait_ge(sem_st, 16 * NCHUNK)

        def gpsimd_body(gpsimd: bass.BassEngine):
            gpsimd.wait_ge(sem_idx, 16)
            for c in range(NCHUNK):
                for k in range(K):
                    m = c * K + k
                    gpsimd.indirect_dma_start(
                        out=g_t[:, m * H : (m + 1) * H],
                        out_offset=None,
                        in_=table[:],
                        in_offset=bass.IndirectOffsetOnAxis(
                            ap=idx_t[:, m : m + 1], axis=0
                        ),
                    ).then_inc(gather_sems[m], 16)

        def vector_body(vector: bass.BassEngine):
            # one-time gate on the weight load; afterwards each op's single
            # wait slot is spent on its gather sem
            vector.wait_ge(sem_w, 16)
            for c in range(NCHUNK):
                m0, m1 = c * K, c * K + 1
                w0 = w_t[:, m0 : m0 + 1]
                w1 = w_t[:, m1 : m1 + 1]
                vector.tensor_scalar(
                    out=acc_t[:],
                    in0=g_t[:, m0 * H : (m0 + 1) * H],
                    scalar1=w0,
                    scalar2=None,
                    op0=mybir.AluOpType.mult,
                )._wait_ge(gather_sems[m0], 16).then_inc(sem_v, 1)
                vector.scalar_tensor_tensor(
                    out=ot_t[:, c * H : (c + 1) * H],
                    in0=g_t[:, m1 * H : (m1 + 1) * H],
                    scalar=w1,
                    in1=acc_t[:],
                    op0=mybir.AluOpType.mult,
                    op1=mybir.AluOpType.add,
                )._wait_ge(gather_sems[m1], 16).then_inc(sem_v, 1)

        # Emit every engine's stream directly into the entry basic block: no
        # per-engine body blocks means no branches, so the sequencers never
        # stall on an IRAM block fetch (~2.5us observed), and there is no
        # end-of-block drain/barrier either.
        sync_body(nc.sync)
        gpsimd_body(nc.gpsimd)
        vector_body(nc.vector)

    # Strip the preamble's const-tile memsets and the post-init all-engine
    # barrier (~2.5us): this kernel never reads the const APs, and each
    # engine's register init precedes its user code in program order anyway.
    entry = nc.m.functions[0].blocks[0]
    drop = {
        ins.name
        for ins in entry.instructions
        if type(ins).__name__
        in ("InstMemset", "InstDrain", "InstEventSemaphore", "InstRegisterMove")
    }
    kept = [ins for ins in entry.instructions if ins.name not in drop]
    del entry.instructions[:]
    for ins in kept:
        entry.instructions.append(ins)

    nc.finalize()
    return nc


def _prepare_in_maps(expert_indices, expert_weights, expert_outputs):
    eo = np.ascontiguousarray(np.asarray(expert_outputs, dtype=np.float32)).reshape(
        E, T, H
    )
    flat_idx = np.asarray(expert_indices).reshape(T, K).astype(np.int32)
    flat_w = np.asarray(expert_weights, dtype=np.float32).reshape(T, K)
    t_local = np.arange(TC, dtype=np.int32)[:, None]
    in_maps = []
    for i in range(N_CORES):
        t0 = i * TC
        slab = np.ascontiguousarray(eo[:, t0 : t0 + TC, :]).reshape(E * TC, H)
        li = flat_idx[t0 : t0 + TC] * TC + t_local  # [TC, K] row idx into slab
        # chunk-major: partition p of chunk c holds token c*128+p
        li = np.ascontiguousarray(
            li.reshape(NCHUNK, P, K).transpose(1, 0, 2).reshape(P, NCHUNK * K)
        )
        w = np.ascontiguousarray(
            flat_w[t0 : t0 + TC]
            .reshape(NCHUNK, P, K)
            .transpose(1, 0, 2)
            .reshape(P, NCHUNK * K)
            .astype(np.float32)
        )
        in_maps.append({"table": slab, "idx": li, "wgt": w})
    return in_maps


def run(
    hidden_states,
    expert_indices,
    expert_weights,
    expert_outputs,
    trace=False,
):
    in_maps = _prepare_in_maps(expert_indices, expert_weights, expert_outputs)
    nc = _build()
    res = run_bass_kernel_spmd(nc, in_maps, list(range(N_CORES)), trace=trace)
    outs = [np.asarray(res.results[i]["out"]) for i in range(N_CORES)]
    full = np.concatenate(outs, axis=0).reshape(B, S, H).astype(np.float32)
    return full, res


def kernel(hidden_states, expert_indices, expert_weights, expert_outputs):
    full, _ = run(hidden_states, expert_indices, expert_weights, expert_outputs)
    return full
